# revision 1
# baseline (speedup 1.0000x reference)
"""BAG-LSTM fused kernel for Trainium2 (Bass/Tile), data-parallel over 8 cores.

v2 (from the 948us baseline):
- All GEMMs f32r (bf16/fp8 fail the 2e-2 gate: measured 1.2-1.5e-2 / 0.2 in sim).
- Strassen-style halving of the BAG W_mb GEMM: host ships
  W_s=(W1+W2)/2, W_d=(W1-W2)/2; device computes s=(ct_a+ct_v)@W_s,
  d=(ct_a-ct_v)@W_d; u1=s+d, u2=s-d. Halves mb-GEMM PE time.
- f32r transposes (1.5 c/row vs f32's 2.0) via bitcast loads + f32r identity.
- o-gate evac also applies the rnn-mask blend (spills o~ = o*m+(1-m)), so the
  BAG h-tail is one multiply.
- ln_g/ln_b are ones/zeros by problem spec -> LayerNorm affine skipped.
- BAG chain spread across DVE/Pool/ACT, sqrts batched [128,4]/[128,2] to
  limit ACT table swaps (sqrt lives in a different act-function set).
- BAG weight loads spread across queues; W_s prefetched during lstm_v.

Layout (per core, batch shard BL=1024 rows): batch on partitions, features on
the free dim. LSTM streams a_W/v_W once in 512-wide gate slabs; c / o~ / c^T
spill to DRAM scratch between phases; BAG reloads them per m-tile.
"""
import sys

import numpy as np

try:
    import concourse.bacc as bacc
except ImportError:  # fresh-dir grading: repo comes from the container env
    sys.path.insert(0, "/opt/trn_rl_repo")
    import concourse.bacc as bacc

import concourse.mybir as mybir
import concourse.tile as tile
from concourse.bass_utils import run_bass_kernel_spmd
from concourse.masks import make_identity
from contextlib import ExitStack

F32 = mybir.dt.float32
F32R = mybir.dt.float32r
F16 = mybir.dt.float16
Act = mybir.ActivationFunctionType
Alu = mybir.AluOpType

NCORES = 8
B, H = 8192, 1024
BL = B // NCORES          # 1024 batch rows per core
MT = BL // 128            # 8 m-tiles
KT1 = H // 128            # 8  k-tiles for H contraction
KT2 = 2 * H // 128        # 16 k-tiles for 2H contraction
LN_EPS = 1e-5
BAG_EPS = 1e-6


def build():
    nc = bacc.Bacc("TRN2", target_bir_lowering=False, debug=False)

    def din(name, shape, dt=F32):
        return nc.dram_tensor(name, shape, dt, kind="ExternalInput")

    def dout(name, shape):
        return nc.dram_tensor(name, shape, F32, kind="ExternalOutput")

    a_x, a_h0 = din("a_x", [BL, H], F16), din("a_h0", [BL, H], F16)
    v_x, v_h0 = din("v_x", [BL, H], F16), din("v_h0", [BL, H], F16)
    a_c0, v_c0 = din("a_c0", [BL, H]), din("v_c0", [BL, H])
    aco = din("aco_is_rnn_list", [BL, 1])
    vis = din("vis_is_rnn_list", [BL, 1])
    isb = din("is_bag_list", [BL, 1])
    a_W, a_b = din("a_W", [2 * H, 4 * H], F16), din("a_b", [4 * H])
    v_W, v_b = din("v_W", [2 * H, 4 * H], F16), din("v_b", [4 * H])
    W_s, W_d = din("W_s", [H, H], F16), din("W_d", [H, H], F16)
    b_mb = din("b_mb", [H], F16)
    W_b, b_b = din("W_b", [H, H], F16), din("b_b", [H], F16)

    a_h, a_sc = dout("a_h", [BL, H]), dout("a_sc", [BL, H])
    v_h, v_sc = dout("v_h", [BL, H]), dout("v_sc", [BL, H])

    # DRAM scratch (per core)
    c_scr = {k: nc.dram_tensor(f"c_{k}_scr", [BL, H], F32R) for k in ("a", "v")}
    o_scr = {k: nc.dram_tensor(f"o_{k}_scr", [BL, H], F32) for k in ("a", "v")}
    ct_scr = {k: nc.dram_tensor(f"ct_{k}_scr", [128, KT1, MT, 128], F16)
              for k in ("a", "v")}
    cts_scr = nc.dram_tensor("cts_scr", [128, KT1, MT, 128], F16)
    ctd_scr = nc.dram_tensor("ctd_scr", [128, KT1, MT, 128], F16)

    with tile.TileContext(nc) as tc, ExitStack() as ctx:
        consts = ctx.enter_context(tc.tile_pool(name="consts", bufs=1))
        stats = ctx.enter_context(tc.tile_pool(name="stats", bufs=24))

        ident_f = consts.tile([128, 128], F32)
        make_identity(nc, ident_f)
        ident = consts.tile([128, 128], F32R)
        nc.vector.tensor_copy(out=ident[:], in_=ident_f[:])
        ident_h = consts.tile([128, 128], F16)
        nc.vector.tensor_copy(out=ident_h[:], in_=ident_f[:])
        ones_f = consts.tile([1, 128], F32)
        nc.vector.memset(ones_f[:], 1.0)
        ones = consts.tile([1, 128], F16)
        nc.vector.tensor_copy(out=ones[:], in_=ones_f[:])

        # per-partition masks [128, MT]: column m = batch rows m*128..m*128+127
        def load_mask(dram):
            t = consts.tile([128, MT], F32, tag=f"mask_{dram.name}")
            nc.sync.dma_start(out=t[:], in_=dram[:].rearrange("(m p) o -> p (m o)", p=128))
            return t

        aco_m = load_mask(aco)
        vis_m = load_mask(vis)
        isb_m = load_mask(isb)
        aco_om = consts.tile([128, MT], F32, tag="aco_om")
        vis_om = consts.tile([128, MT], F32, tag="vis_om")
        nc.vector.tensor_scalar(out=aco_om[:], in0=aco_m[:], scalar1=-1.0,
                                scalar2=1.0, op0=Alu.mult, op1=Alu.add)
        nc.vector.tensor_scalar(out=vis_om[:], in0=vis_m[:], scalar1=-1.0,
                                scalar2=1.0, op0=Alu.mult, op1=Alu.add)

        isb_om = consts.tile([128, MT], F32, tag="isb_om")
        nc.vector.tensor_scalar(out=isb_om[:], in0=isb_m[:], scalar1=-1.0,
                                scalar2=1.0, op0=Alu.mult, op1=Alu.add)
        epsb = consts.tile([128, 1], F32, tag="epsb")
        nc.vector.memset(epsb[:], BAG_EPS)
        epsl = consts.tile([128, 1], F32, tag="epsl")
        nc.vector.memset(epsl[:], LN_EPS)
        # ||c||^2 per cell, [128, MT] resident across phases
        ems_res = {}
        for k in ("a", "v"):
            ems_t = consts.tile([128, MT], F32, tag=f"ems_{k}")
            ems_res[k] = ems_t

        # ---------------- LSTM phase (run twice: a then v) ----------------
        def lstm_phase(tag, x_in, h0_in, c0_in, W_in, b_in, m_col, om_col,
                       wbufs=4, sumdiff=False):
            with ExitStack() as ph:
                xtp = ph.enter_context(tc.tile_pool(name=f"xt_{tag}", bufs=1))
                wlp = ph.enter_context(tc.tile_pool(name=f"wl_{tag}", bufs=wbufs))
                xrp = ph.enter_context(tc.tile_pool(name=f"xr_{tag}", bufs=4))
                pap = ph.enter_context(tc.tile_pool(name=f"pa_{tag}", bufs=1))
                c0p = ph.enter_context(tc.tile_pool(name=f"c0_{tag}", bufs=2))
                gep = ph.enter_context(tc.tile_pool(name=f"ge_{tag}", bufs=3))
                ccp = ph.enter_context(tc.tile_pool(name=f"cc_{tag}", bufs=2))
                ctev = ph.enter_context(tc.tile_pool(name=f"ctv_{tag}", bufs=2))
                sdp = ph.enter_context(tc.tile_pool(name=f"sd_{tag}", bufs=2))
                bp = ph.enter_context(tc.tile_pool(name=f"bp_{tag}", bufs=2))
                sqp = ph.enter_context(tc.tile_pool(name=f"sq_{tag}", bufs=2))
                gps = ph.enter_context(tc.tile_pool(name=f"gp_{tag}", bufs=6,
                                                    space="PSUM"))
                tps = ph.enter_context(tc.tile_pool(name=f"tp_{tag}", bufs=2,
                                                    space="PSUM"))

                with nc.named_scope(f"xt_{tag}"):
                    # X.T tiles: k 0..7 from x, 8..15 from h0 (fp16 transpose)
                    xt = xtp.tile([128, KT2, MT, 128], F16, tag="xt")
                    for src, kofs in ((x_in, 0), (h0_in, KT1)):
                        for m in range(MT):
                            xr = xrp.tile([128, H], F16, tag="xrow")
                            nc.scalar.dma_start(out=xr[:],
                                                in_=src[m * 128:(m + 1) * 128, :])
                            for k in range(KT1):
                                tp = tps.tile([128, 128], F16, tag="tp")
                                nc.tensor.transpose(
                                    tp[:], xr[:, k * 128:(k + 1) * 128],
                                    ident_h[:])
                                if k % 2 == 0:
                                    nc.scalar.copy(out=xt[:, kofs + k, m, :],
                                                   in_=tp[:])
                                else:
                                    nc.vector.tensor_copy(
                                        out=xt[:, kofs + k, m, :], in_=tp[:])

                with nc.named_scope(f"lstm_{tag}"):
                    for ns in range(2):
                        pacc = pap.tile([128, MT, 512], F32, tag="pacc")
                        for gate in (0, 2, 1, 3):      # i, g, f, o
                            cols = gate * H + ns * 512
                            wt_lo = wlp.tile([128, KT1, 512], F16, tag="wslab")
                            nc.gpsimd.dma_start(
                                out=wt_lo[:],
                                in_=W_in[:H, cols:cols + 512].rearrange(
                                    "(k p) c -> p k c", p=128))
                            wt_hi = wlp.tile([128, KT1, 512], F16, tag="wslab")
                            nc.gpsimd.dma_start(
                                out=wt_hi[:],
                                in_=W_in[H:, cols:cols + 512].rearrange(
                                    "(k p) c -> p k c", p=128))
                            bt = bp.tile([128, 512], F32, tag="brow")
                            nc.sync.dma_start(
                                out=bt[:],
                                in_=b_in[cols:cols + 512].unsqueeze(0)
                                .partition_broadcast(128).squeeze(1))
                            for m in range(MT):
                                pt = gps.tile([128, 512], F32, tag="gpt")
                                for k in range(KT2):
                                    wsrc = wt_lo if k < KT1 else wt_hi
                                    nc.tensor.matmul(pt[:], xt[:, k, m, :],
                                                     wsrc[:, k % KT1, :],
                                                     start=(k == 0),
                                                     stop=(k == KT2 - 1))
                                gb = gep.tile([128, 512], F32, tag="gb")
                                nc.vector.tensor_add(gb[:], pt[:], bt[:])
                                if gate == 0:          # i -> P
                                    nc.scalar.activation(out=pacc[:, m, :],
                                                         in_=gb[:],
                                                         func=Act.Sigmoid)
                                elif gate == 2:        # g: P *= tanh(g)
                                    nc.scalar.activation(out=gb[:], in_=gb[:],
                                                         func=Act.Tanh)
                                    nc.vector.tensor_mul(pacc[:, m, :],
                                                         pacc[:, m, :], gb[:])
                                elif gate == 1:        # f: finish c
                                    nc.scalar.activation(out=gb[:], in_=gb[:],
                                                         func=Act.Sigmoid)
                                    nc.vector.tensor_scalar(
                                        out=gb[:], in0=gb[:],
                                        scalar1=m_col[:, m:m + 1],
                                        scalar2=om_col[:, m:m + 1],
                                        op0=Alu.mult, op1=Alu.add)
                                    c0b = c0p.tile([128, 512], F32, tag="c0b")
                                    nc.sync.dma_start(
                                        out=c0b[:],
                                        in_=c0_in[m * 128:(m + 1) * 128,
                                                  ns * 512:(ns + 1) * 512])
                                    nc.vector.tensor_mul(gb[:], gb[:], c0b[:])
                                    cb = ccp.tile([128, 512], F32R, tag="cb")
                                    nc.vector.scalar_tensor_tensor(
                                        out=cb[:], in0=pacc[:, m, :],
                                        scalar=m_col[:, m:m + 1], in1=gb[:],
                                        op0=Alu.mult, op1=Alu.add)
                                    nc.sync.dma_start(
                                        out=c_scr[tag][m * 128:(m + 1) * 128,
                                                       ns * 512:(ns + 1) * 512],
                                        in_=cb[:])
                                    # ||c||^2 partial (ACT square w/ accum)
                                    sqj = sqp.tile([128, 512], F32, tag="sqj")
                                    emsp = stats.tile([128, 1], F32, tag="emsp")
                                    nc.scalar.activation(out=sqj[:], in_=cb[:],
                                                         func=Act.Square,
                                                         accum_out=emsp[:])
                                    if ns == 0:
                                        nc.vector.tensor_copy(
                                            out=ems_res[tag][:, m:m + 1],
                                            in_=emsp[:])
                                    else:
                                        nc.vector.tensor_add(
                                            ems_res[tag][:, m:m + 1],
                                            ems_res[tag][:, m:m + 1], emsp[:])
                                    cth = ctev.tile([128, 4, 128], F16,
                                                    tag="cth")
                                    for hh in range(4):
                                        tp = tps.tile([128, 128], F32R, tag="tp")
                                        nc.tensor.transpose(
                                            tp[:],
                                            cb[:, hh * 128:(hh + 1) * 128],
                                            ident[:])
                                        nc.scalar.copy(out=cth[:, hh, :], in_=tp[:])
                                    nc.scalar.dma_start(
                                        out=ct_scr[tag][:, ns * 4:ns * 4 + 4, m, :],
                                        in_=cth[:])
                                    if sumdiff:
                                        ctah = sdp.tile([128, 4, 128], F16,
                                                        tag="ctah")
                                        nc.scalar.dma_start(
                                            out=ctah[:],
                                            in_=ct_scr["a"][:, ns * 4:ns * 4 + 4,
                                                            m, :])
                                        ctsh = sdp.tile([128, 4, 128], F16,
                                                        tag="ctsh")
                                        nc.vector.tensor_add(ctsh[:], ctah[:],
                                                             cth[:])
                                        nc.scalar.dma_start(
                                            out=cts_scr[:, ns * 4:ns * 4 + 4, m, :],
                                            in_=ctsh[:])
                                        ctdh = sdp.tile([128, 4, 128], F16,
                                                        tag="ctdh")
                                        nc.vector.tensor_sub(ctdh[:], ctah[:],
                                                             cth[:])
                                        nc.scalar.dma_start(
                                            out=ctd_scr[:, ns * 4:ns * 4 + 4, m, :],
                                            in_=ctdh[:])
                                else:                  # o: spill o~ = o*m+(1-m)
                                    nc.scalar.activation(out=gb[:], in_=gb[:],
                                                         func=Act.Sigmoid)
                                    nc.vector.tensor_scalar(
                                        out=gb[:], in0=gb[:],
                                        scalar1=m_col[:, m:m + 1],
                                        scalar2=om_col[:, m:m + 1],
                                        op0=Alu.mult, op1=Alu.add)
                                    nc.gpsimd.dma_start(
                                        out=o_scr[tag][m * 128:(m + 1) * 128,
                                                       ns * 512:(ns + 1) * 512],
                                        in_=gb[:])

        lstm_phase("a", a_x, a_h0, a_c0, a_W, a_b, aco_m, aco_om)

        with ExitStack() as phb:
            # fp16 bag weights (48KB) prefetched during lstm_v: the pool
            # lands on phase-a regions, so loads start as lstm_a drains.
            bwp = phb.enter_context(tc.tile_pool(name="bagw", bufs=1))
            ws_t = bwp.tile([128, KT1, H], F16, tag="wst")
            wd_t = bwp.tile([128, KT1, H], F16, tag="wdt")
            wb_t = bwp.tile([128, KT1, H], F16, tag="wbt")
            for k in range(KT1):
                nc.sync.dma_start(out=ws_t[:, k, :],
                                  in_=W_s[k * 128:(k + 1) * 128, :])
                nc.sync.dma_start(out=wd_t[:, k, :],
                                  in_=W_d[k * 128:(k + 1) * 128, :])
                nc.sync.dma_start(out=wb_t[:, k, :],
                                  in_=W_b[k * 128:(k + 1) * 128, :])
            bmb = []
            bbt = []
            for r in range(2):
                t1 = bwp.tile([1, 512], F16, tag=f"bmb{r}")
                nc.sync.dma_start(
                    out=t1[:], in_=b_mb[r * 512:(r + 1) * 512].unsqueeze(0))
                bmb.append(t1)
                t2 = bwp.tile([1, 512], F16, tag=f"bbt{r}")
                nc.sync.dma_start(
                    out=t2[:], in_=b_b[r * 512:(r + 1) * 512].unsqueeze(0))
                bbt.append(t2)

            lstm_phase("v", v_x, v_h0, v_c0, v_W, v_b, vis_m, vis_om, wbufs=4,
                       sumdiff=True)

            # ---------------- BAG phase ----------------
            with ExitStack() as ph:
                # pool open order targets early-freed lstm_v stack regions:
                # wd_t lands on xr/pacc (freed ~f-gate), streams on c0/cc/sq.
                ctp = ph.enter_context(tc.tile_pool(name="bagct", bufs=2))
                csp = ph.enter_context(tc.tile_pool(name="bagcs", bufs=2))
                cmp_ = ph.enter_context(tc.tile_pool(name="bagcm", bufs=2))
                orp = ph.enter_context(tc.tile_pool(name="bagor", bufs=2))
                wbp = ph.enter_context(tc.tile_pool(name="bagwb", bufs=1))
                hmp = ph.enter_context(tc.tile_pool(name="baghm", bufs=2))
                jkp = ph.enter_context(tc.tile_pool(name="bagjk", bufs=2))
                ubp = ph.enter_context(tc.tile_pool(name="bagub", bufs=1))
                bps = ph.enter_context(tc.tile_pool(name="bagps", bufs=1,
                                                    space="PSUM"))

                # sqrt(||c||^2) for all m once (saves per-m copies + sqrt width)
                emn_res = {}
                for cell in ("a", "v"):
                    emn_t = consts.tile([128, MT], F32, tag=f"emn_{cell}")
                    nc.scalar.activation(out=emn_t[:], in_=ems_res[cell][:],
                                         func=Act.Sqrt)
                    emn_res[cell] = emn_t

                def bag_load(m):
                    st = {}
                    st["cta"] = ctp.tile([128, KT1, 128], F16, name="cta", tag="cta")
                    nc.sync.dma_start(out=st["cta"][:], in_=ct_scr["a"][:, :, m, :])
                    st["ctv"] = ctp.tile([128, KT1, 128], F16, name="ctv", tag="ctv")
                    nc.scalar.dma_start(out=st["ctv"][:], in_=ct_scr["v"][:, :, m, :])
                    st["cts"] = csp.tile([128, KT1, 128], F16, name="cts", tag="cts")
                    nc.gpsimd.dma_start(out=st["cts"][:], in_=cts_scr[:, :, m, :])
                    st["ctd"] = csp.tile([128, KT1, 128], F16, name="ctd", tag="ctd")
                    nc.sync.dma_start(out=st["ctd"][:], in_=ctd_scr[:, :, m, :])
                    st["ca"] = cmp_.tile([128, H], F32R, name="ca", tag="ca")
                    nc.gpsimd.dma_start(out=st["ca"][:],
                                        in_=c_scr["a"][m * 128:(m + 1) * 128, :])
                    st["cv"] = cmp_.tile([128, H], F32R, name="cv", tag="cv")
                    nc.gpsimd.dma_start(out=st["cv"][:],
                                        in_=c_scr["v"][m * 128:(m + 1) * 128, :])
                    st["oa"] = orp.tile([128, H], F32, name="oa", tag="oa")
                    nc.sync.dma_start(out=st["oa"][:],
                                      in_=o_scr["a"][m * 128:(m + 1) * 128, :])
                    st["ov"] = orp.tile([128, H], F32, name="ov", tag="ov")
                    nc.scalar.dma_start(out=st["ov"][:],
                                        in_=o_scr["v"][m * 128:(m + 1) * 128, :])
                    return st

                def bag_gemms(st):
                    ps = {}
                    for name, st_src, wsrc, bias in (
                            ("s", st["cts"], ws_t, bmb),
                            ("d", st["ctd"], wd_t, None),
                            ("w1", st["ctv"], wb_t, bbt),
                            ("w2", st["cta"], wb_t, bbt)):
                        for nsh in range(2):
                            p = bps.tile([128, 512], F32, tag=f"ps_{name}{nsh}")
                            for k in range(KT1):
                                nc.tensor.matmul(
                                    p[:], st_src[:, k, :],
                                    wsrc[:, k, nsh * 512:(nsh + 1) * 512],
                                    start=(k == 0),
                                    stop=(k == KT1 - 1 and bias is None))
                            if bias is not None:
                                nc.tensor.matmul(p[:], ones[:], bias[nsh][:],
                                                 start=False, stop=True)
                            ps[f"{name}{nsh}"] = p
                    return ps

                def bag_part1(st, ps):
                    # everything that reads PSUM: evacuate so the next m's
                    # GEMMs can reuse the banks.
                    dw = stats.tile([128, 1], F32, tag="dw")
                    nc.scalar.activation(out=dw[:], in_=epsb[:], func=Act.Sqrt)
                    u1 = ubp.tile([128, H], F32, tag="u1")
                    u2 = ubp.tile([128, H], F32, tag="u2")
                    st["u1"], st["u2"] = u1, u2
                    for nsh in range(2):
                        sl = slice(nsh * 512, (nsh + 1) * 512)
                        nc.scalar.copy(out=u1[:, sl], in_=ps[f"s{nsh}"][:])
                        nc.vector.tensor_sub(u2[:, sl], u1[:, sl],
                                             ps[f"d{nsh}"][:])
                        nc.vector.tensor_add(u1[:, sl], u1[:, sl],
                                             ps[f"d{nsh}"][:])
                    wb1 = wbp.tile([128, H], F32, tag="wb1")
                    nc.scalar.activation(out=wb1[:], in_=u1[:], func=Act.Relu)
                    wb2 = wbp.tile([128, H], F32, tag="wb2")
                    nc.scalar.activation(out=wb2[:], in_=u2[:], func=Act.Relu)
                    hm1 = hmp.tile([128, H], F32, tag="hm1")
                    hm2 = hmp.tile([128, H], F32, tag="hm2")
                    for nsh in range(2):
                        sl = slice(nsh * 512, (nsh + 1) * 512)
                        nc.vector.tensor_mul(hm1[:, sl], wb1[:, sl],
                                             ps[f"w1{nsh}"][:])
                        nc.vector.tensor_mul(hm2[:, sl], wb2[:, sl],
                                             ps[f"w2{nsh}"][:])
                    st["hm1"], st["hm2"] = hm1, hm2

                def bag_part2(m, st):
                    hm1, hm2 = st["hm1"], st["hm2"]
                    ca, cv = st["ca"], st["cv"]
                    st8 = stats.tile([128, 8], F32, tag="st8")
                    norms = stats.tile([128, 2], F32, tag="norms")
                    nc.scalar.activation(out=st["u1"][:], in_=hm1[:],
                                         func=Act.Square, accum_out=st8[:, 2:3])
                    nc.scalar.activation(out=st["u2"][:], in_=hm2[:],
                                         func=Act.Square, accum_out=st8[:, 3:4])
                    # hmn = sqrt(hms) (table already warmed by part1 dummy)
                    nc.scalar.activation(out=norms[:], in_=st8[:, 2:4],
                                         func=Act.Sqrt)
                    alph = stats.tile([128, 2], F32, tag="alph")
                    hre = stats.tile([128, 2], F32, tag="hre")
                    nc.vector.tensor_scalar(out=hre[:], in0=norms[:],
                                            scalar1=epsb[:], scalar2=None,
                                            op0=Alu.add, op1=Alu.bypass)
                    nc.vector.reciprocal(out=hre[:], in_=hre[:])
                    nc.vector.tensor_mul(alph[:, 0:1], emn_res["a"][:, m:m + 1],
                                         hre[:, 0:1])
                    nc.vector.tensor_mul(alph[:, 1:2], emn_res["v"][:, m:m + 1],
                                         hre[:, 1:2])
                    nc.vector.tensor_scalar_min(alph[:], alph[:], 1.0)
                    # pre = alpha*hm + main  (accum -> s1)
                    nc.vector.scalar_tensor_tensor(
                        out=hm1[:], in0=hm1[:], scalar=alph[:, 0:1], in1=ca[:],
                        op0=Alu.mult, op1=Alu.add, accum_out=st8[:, 4:5])
                    nc.vector.scalar_tensor_tensor(
                        out=hm2[:], in0=hm2[:], scalar=alph[:, 1:2], in1=cv[:],
                        op0=Alu.mult, op1=Alu.add, accum_out=st8[:, 5:6])
                    nc.scalar.activation(out=st["u1"][:], in_=hm1[:],
                                         func=Act.Square, accum_out=st8[:, 6:7])
                    nc.scalar.activation(out=st["u2"][:], in_=hm2[:],
                                         func=Act.Square, accum_out=st8[:, 7:8])
                    nmu = stats.tile([128, 2], F32, tag="nmu")
                    nc.vector.tensor_scalar_mul(nmu[:], st8[:, 4:6], -1.0 / H)
                    var = stats.tile([128, 2], F32, tag="var")
                    nc.vector.tensor_scalar_mul(var[:], st8[:, 6:8], 1.0 / H)
                    mu2 = stats.tile([128, 2], F32, tag="mu2")
                    nc.vector.tensor_mul(mu2[:], nmu[:], nmu[:])
                    nc.vector.tensor_sub(var[:], var[:], mu2[:])
                    rstd = stats.tile([128, 2], F32, tag="rstd")
                    nc.scalar.activation(out=rstd[:], in_=var[:], func=Act.Sqrt,
                                         bias=epsl[:], scale=1.0)
                    # warm the sigmoid/tanh table while DVE finishes the LN
                    dw2 = stats.tile([128, 1], F32, tag="dw2")
                    nc.scalar.activation(out=dw2[:], in_=epsb[:], func=Act.Tanh)
                    nc.vector.reciprocal(out=rstd[:], in_=rstd[:])
                    # rs2 = rstd*is_bag folds the blend into the LN apply:
                    # shift = main*(1-isb) + (pre+nmu)*rstd*isb
                    rs2 = stats.tile([128, 2], F32, tag="rs2")
                    nc.vector.tensor_mul(rs2[:, 0:1], rstd[:, 0:1],
                                         isb_m[:, m:m + 1])
                    nc.vector.tensor_mul(rs2[:, 1:2], rstd[:, 1:2],
                                         isb_m[:, m:m + 1])

                    for hm, main, col, out_sc, out_h, o_t in (
                            (hm1, ca, 0, a_sc, a_h, st["oa"]),
                            (hm2, cv, 1, v_sc, v_h, st["ov"])):
                        nc.vector.tensor_scalar(
                            out=hm[:], in0=hm[:], scalar1=nmu[:, col:col + 1],
                            scalar2=rs2[:, col:col + 1],
                            op0=Alu.add, op1=Alu.mult)
                        sh = jkp.tile([128, H], F32, tag="sh")
                        nc.vector.scalar_tensor_tensor(
                            out=sh[:], in0=main[:], scalar=isb_om[:, m:m + 1],
                            in1=hm[:], op0=Alu.mult, op1=Alu.add)
                        nc.sync.dma_start(out=out_sc[m * 128:(m + 1) * 128, :],
                                          in_=sh[:])
                        th = jkp.tile([128, H], F32, tag="th")
                        nc.scalar.activation(out=th[:], in_=sh[:], func=Act.Tanh)
                        hh = jkp.tile([128, H], F32, tag="hh")
                        nc.vector.tensor_mul(hh[:], o_t[:], th[:])
                        nc.gpsimd.dma_start(
                            out=out_h[m * 128:(m + 1) * 128, :], in_=hh[:])

                with nc.named_scope("bag"):
                    # software-pipelined: GEMMs(m) are emitted between
                    # part1(m-1) (PSUM evac) and part2(m-1), and the Pool
                    # sum/diff for m comes before part1(m-1) in the FIFO.
                    prev = None
                    st = bag_load(0)
                    ps = bag_gemms(st)
                    prev = (0, st, ps)
                    for m in range(1, MT):
                        stn = bag_load(m)
                        pm, pst, pps = prev
                        bag_part1(pst, pps)
                        if m == MT - 1:
                            # last tile: chain(m-1) fully before the final
                            # GEMMs (no later GEMM to stall) -> shorter tail
                            bag_part2(pm, pst)
                        psn = bag_gemms(stn)
                        if m != MT - 1:
                            bag_part2(pm, pst)
                        prev = (m, stn, psn)
                    pm, pst, pps = prev
                    bag_part1(pst, pps)
                    bag_part2(pm, pst)

    nc.compile()
    return nc


_NC = None


def _get_nc():
    global _NC
    if _NC is None:
        _NC = build()
    return _NC


BATCH_INPUTS = ("a_x", "a_h0", "a_c0", "v_x", "v_h0", "v_c0",
                "aco_is_rnn_list", "vis_is_rnn_list", "is_bag_list")
F16_INPUTS = ("a_x", "a_h0", "v_x", "v_h0", "a_W", "v_W",
              "W_s", "W_d", "W_b", "b_mb", "b_b")


def _round_f32r(a):
    """Exact float32r rounding (fp32 with 11 explicit mantissa bits, RNE) —
    bitwise-identical to the on-chip DMA/DVE cast (verified on HW)."""
    b = np.ascontiguousarray(a, dtype=np.float32).view(np.uint32)
    lsb = (b >> np.uint32(12)) & np.uint32(1)
    r = (b + np.uint32((1 << 11) - 1) + lsb) & np.uint32(0xFFFFF000)
    return r.view(np.float32)


def prepare_in_maps(inputs):
    prep = {k: np.ascontiguousarray(np.asarray(v), dtype=np.float32)
            for k, v in inputs.items()}
    W_mb = prep.pop("W_mb").astype(np.float64)
    prep["W_s"] = ((W_mb[:H] + W_mb[H:]) * 0.5).astype(np.float32)
    prep["W_d"] = ((W_mb[:H] - W_mb[H:]) * 0.5).astype(np.float32)
    prep.pop("ln_g"), prep.pop("ln_b")  # identity by problem spec
    for k in F16_INPUTS:
        prep[k] = prep[k].astype(np.float16)
    in_maps = []
    for c in range(NCORES):
        im = {}
        for k, v in prep.items():
            im[k] = v[c * BL:(c + 1) * BL] if k in BATCH_INPUTS else v
        in_maps.append(im)
    return in_maps


def kernel(**inputs):
    nc = _get_nc()
    in_maps = prepare_in_maps(inputs)
    res = run_bass_kernel_spmd(nc, in_maps, list(range(NCORES)))
    outs = res.results
    cat = lambda name: np.concatenate([outs[c][name] for c in range(NCORES)], axis=0)
    return (cat("a_h"), cat("a_sc"), cat("v_h"), cat("v_sc"))

